# revision 3
# baseline (speedup 1.0000x reference)
# Causal self-attention (B=2, T=2048, D=1024, H=16, HD=64) with RoPE on 8 TRN2
# cores.
#
# Sharding: data-parallel over batch (2 groups of 4 cores), tensor-parallel
# over heads within each group (4 heads per core, as 2 head-pairs p=0,1).
# Everything on-device is bf16 (inputs pre-converted on host): bf16 matmuls run
# at 1 cycle/row at ANY moving size (no fp32r >=256 constraint), DVE
# elementwise ops get the 2x packed mode, and DMA bytes are halved.
#
# Per core:
#   Phase A - single t-chunk-major sweep over x: for each 512-col chunk of T,
#     accumulate all six 128-col qkv projections (q/k/v x 2 pairs) over the 8
#     contraction tiles, then drain: RoPE (Act copy + DVE shuffle/mul/mul/add)
#     for q/k, PE transposes + Act copies into the AV-stationary layout for v
#     (with an appended ones column producing the softmax denominator free).
#   Attention - per 512-row q strip and head pair: S^T blocks (k-block x q)
#     with S(kb+1) emitted before AV(kb) so the PE never waits on exp; exp on
#     the Scalar engine (both heads in one call), causal mask via
#     gpsimd.affine_select on diagonal blocks only, AV accumulating [65, q]
#     (row 64 = denominator), then reciprocal/broadcast/mul into oT.
#   Out-projection - row-sharded partial [D, T]; interleaved one strip behind
#     attention, reusing the S-psum tag so PSUM stays within 8 banks. Host
#     sums the 4 partials per batch and transposes back.
import sys
import os

sys.path.insert(0, "/opt/trn_rl_repo")

import numpy as np

import concourse.bass as bass  # noqa: F401  (bass types used via bacc)
import concourse.mybir as mybir
from concourse import bacc
from concourse.tile import TileContext
from concourse.bass_utils import run_bass_kernel_spmd
from contextlib import ExitStack

F32 = mybir.dt.float32
BF16 = mybir.dt.bfloat16
AF = mybir.ActivationFunctionType
ALU = mybir.AluOpType

B, T, D = 2, 2048, 1024
H, HD = 16, 64
NCORES = 8
GROUPS = NCORES // B          # cores per batch = 4
HPC = H // GROUPS             # heads per core = 4
NK = D // 128                 # contraction tiles for D
SCALE = HD ** -0.5

# hd interleave: new row 2j <- orig j, new row 2j+1 <- orig j+32 so the
# rotate-half partner of every row is its neighbour (swappable by a 32-lane
# stream shuffle).
PI = np.empty(HD, dtype=np.int64)
PI[0::2] = np.arange(32)
PI[1::2] = np.arange(32, 64)

SWAP_MASK = []
for _i in range(16):
    SWAP_MASK += [2 * _i + 1, 2 * _i]

# c-tile order within one (kt, t) group: q0, k0 first so pair-0 RoPE can
# drain earliest; v next so AV stationaries are ready before attention.
C_ORDER = [2, 4, 0, 1, 3, 5]


def _build_program():
    nc = bacc.Bacc("TRN2", target_bir_lowering=False, debug=False,
                   num_devices=NCORES)
    d_x = nc.dram_tensor("xT", [D, T], BF16, kind="ExternalInput").ap()
    d_w = nc.dram_tensor("w_cat", [D, 6 * 128], BF16,
                         kind="ExternalInput").ap()
    d_wo = nc.dram_tensor("w_o", [2 * 128, D], BF16,
                          kind="ExternalInput").ap()
    d_cos = nc.dram_tensor("cos2", [128, T], BF16, kind="ExternalInput").ap()
    d_sin = nc.dram_tensor("sin2", [128, T], BF16, kind="ExternalInput").ap()
    d_id = nc.dram_tensor("ident", [128, 128], BF16,
                          kind="ExternalInput").ap()
    d_ones = nc.dram_tensor("ones16", [128, 16], BF16,
                            kind="ExternalInput").ap()
    d_out = nc.dram_tensor("outp", [D, T], BF16, kind="ExternalOutput").ap()

    with TileContext(nc) as tc, nc.allow_low_precision(reason="bf16 attn"):
        with ExitStack() as root:
            qkv_pool = root.enter_context(tc.tile_pool(name="qkv", bufs=1))
            va_pool = root.enter_context(tc.tile_pool(name="va", bufs=1))
            out_pool = root.enter_context(tc.tile_pool(name="outT", bufs=1))
            wop = root.enter_context(tc.tile_pool(name="wop", bufs=1))
            wu_pool = root.enter_context(tc.tile_pool(name="wu", bufs=1))

            qT = [qkv_pool.tile([128, T], BF16, tag=f"q{p}", name=f"qT{p}")
                  for p in range(2)]
            kT = [qkv_pool.tile([128, T], BF16, tag=f"k{p}", name=f"kTt{p}")
                  for p in range(2)]
            va = [va_pool.tile([128, 16 * 65], BF16, tag=f"va{h}",
                               name=f"va{h}") for h in range(HPC)]
            oT = [out_pool.tile([128, T], BF16, tag=f"o{p}", name=f"oT{p}")
                  for p in range(2)]
            wo_sb = [wop.tile([128, D], BF16, tag=f"wo{p}", name=f"wo{p}")
                     for p in range(2)]

            # Warm the Act engine's exp table before it matters.
            wu = wu_pool.tile([1, 2], F32, tag="wu")
            wu2 = wu_pool.tile([1, 2], F32, tag="wu2")
            nc.vector.memset(wu[:], 0.0)
            nc.scalar.activation(wu2[:], wu[:], AF.Exp, scale=1.0)

            # ---------------- Phase A: qkv projection + RoPE + v transpose
            with nc.named_scope("qkv"):
                with ExitStack() as sA:
                    tab = sA.enter_context(tc.tile_pool(name="tab", bufs=1))
                    xp = sA.enter_context(tc.tile_pool(name="xp", bufs=1))
                    wp = sA.enter_context(tc.tile_pool(name="wp", bufs=1))
                    tp = sA.enter_context(tc.tile_pool(name="ropetmp",
                                                       bufs=2))
                    vtp = sA.enter_context(tc.tile_pool(name="vT", bufs=1))

                    cos2 = tab.tile([128, T], BF16, tag="cos")
                    sin2 = tab.tile([128, T], BF16, tag="sin")
                    ident = tab.tile([128, 128], BF16, tag="id")
                    vT = [vtp.tile([128, T], BF16, tag=f"v{p}",
                                   name=f"vT{p}") for p in range(2)]

                    nc.sync.dma_start(out=ident[:], in_=d_id[:])
                    for h in range(HPC):
                        nc.sync.dma_start(out=va[h][:, 64:16 * 65:65],
                                          in_=d_ones[:])

                    x_sb = [xp.tile([128, T], BF16, tag=f"x{kt}",
                                    name=f"xsb{kt}") for kt in range(NK)]
                    w_sb = {}
                    # interleave x(t=0) chunks with the w tiles each needs so
                    # the PE can start ~1us in.
                    for kt in range(NK):
                        nc.sync.dma_start(out=x_sb[kt][:, 0:512],
                                          in_=d_x[kt * 128:(kt + 1) * 128,
                                                  0:512])
                        for c in C_ORDER:
                            w_t = wp.tile([128, 128], BF16, tag=f"w{kt}_{c}")
                            nc.sync.dma_start(
                                out=w_t[:],
                                in_=d_w[kt * 128:(kt + 1) * 128,
                                        c * 128:(c + 1) * 128])
                            w_sb[(kt, c)] = w_t

                    accp = tc.alloc_tile_pool(name="accs", bufs=1,
                                              space="PSUM")
                    psT = tc.alloc_tile_pool(name="psT", bufs=2, space="PSUM",
                                             side="right")

                    def emit_rope(c, acc, tsl):
                        pair = (c - 2) % 2 if c in (2, 3) else (c - 4) % 2
                        dst = qT[c - 2] if c in (2, 3) else kT[c - 4]
                        del pair
                        qsb = tp.tile([128, 512], BF16, tag="qsb")
                        nc.scalar.copy(qsb[:], acc[:])
                        qsh = tp.tile([128, 512], BF16, tag="qsh")
                        nc.vector.stream_shuffle(qsh[:], acc[:], SWAP_MASK)
                        tcos = tp.tile([128, 512], BF16, tag="tcos")
                        nc.vector.tensor_tensor(out=tcos[:], in0=qsb[:],
                                                in1=cos2[:, tsl],
                                                op=ALU.mult)
                        nc.vector.tensor_tensor(out=qsh[:], in0=qsh[:],
                                                in1=sin2[:, tsl],
                                                op=ALU.mult)
                        nc.vector.tensor_tensor(out=dst[:, tsl], in0=tcos[:],
                                                in1=qsh[:], op=ALU.add)

                    def emit_vtrans(t):
                        # transposes + va copies for both v c-tiles of chunk t
                        for p in range(2):
                            pt_ = psT.tile([128, 512], BF16, tag="pt",
                                           name=f"ptr{p}_{t}")
                            for j in range(4):
                                tt = 4 * t + j
                                nc.tensor.transpose(
                                    pt_[:, j * 128:(j + 1) * 128],
                                    vT[p][:, tt * 128:(tt + 1) * 128],
                                    ident[:])
                            for j in range(4):
                                tt = 4 * t + j
                                nc.scalar.copy(
                                    va[2 * p][:, tt * 65:tt * 65 + 64],
                                    pt_[:, j * 128:j * 128 + 64])
                                nc.scalar.copy(
                                    va[2 * p + 1][:, tt * 65:tt * 65 + 64],
                                    pt_[:, j * 128 + 64:j * 128 + 128])

                    for t in range(4):
                        tsl = slice(t * 512, (t + 1) * 512)
                        accs = {c: accp.tile([128, 512], F32, tag=f"a{c}",
                                             name=f"acc{c}_{t}")
                                for c in C_ORDER}
                        for kt in range(NK):
                            for c in C_ORDER:
                                nc.tensor.matmul(
                                    accs[c][:], w_sb[(kt, c)][:],
                                    x_sb[kt][:, tsl],
                                    start=(kt == 0), stop=(kt == NK - 1))
                        if t > 0:
                            emit_vtrans(t - 1)
                        # stream the tables + next x chunks behind this t's
                        # matmuls
                        nc.sync.dma_start(out=cos2[:, tsl], in_=d_cos[:, tsl])
                        nc.sync.dma_start(out=sin2[:, tsl], in_=d_sin[:, tsl])
                        if t < 3:
                            nsl = slice((t + 1) * 512, (t + 2) * 512)
                            for kt in range(NK):
                                nc.sync.dma_start(
                                    out=x_sb[kt][:, nsl],
                                    in_=d_x[kt * 128:(kt + 1) * 128, nsl])
                        # drains: v copies to SBUF, RoPE for q/k
                        for p in range(2):
                            nc.scalar.copy(vT[p][:, tsl], accs[p][:])
                        for c in (2, 4, 3, 5):
                            emit_rope(c, accs[c], tsl)
                    emit_vtrans(3)
                    psT.release()
                    accp.release()

            # ---------------- attention + interleaved out-projection
            psS = tc.alloc_tile_pool(name="psS", bufs=2, space="PSUM")
            psV = tc.alloc_tile_pool(name="psV", bufs=2, space="PSUM",
                                     side="right")

            with nc.named_scope("attn"):
                with ExitStack() as sB:
                    ptp = sB.enter_context(tc.tile_pool(name="ptp", bufs=6))
                    rp = sB.enter_context(tc.tile_pool(name="rp", bufs=2))
                    fop = sB.enter_context(tc.tile_pool(name="fop", bufs=3))
                    for p in range(2):
                        nc.sync.dma_start(
                            out=wo_sb[p][:],
                            in_=d_wo[p * 128:(p + 1) * 128, :])

                    def emit_strip(si, p):
                        q0 = 512 * si
                        kb_max = 4 * (si + 1)
                        av = psV.tile([65, 1024], F32, tag="av",
                                      name=f"avps{si}_{p}")
                        pend = []      # (kb, o, ptb) waiting for AV emission

                        def emit_av(kb, o, ptb):
                            L = 512 - o
                            for hl in range(2):
                                nc.tensor.matmul(
                                    av[:, hl * 512 + o:hl * 512 + 512],
                                    va[2 * p + hl][:, kb * 65:kb * 65 + 65],
                                    ptb[:, hl * 512:hl * 512 + L],
                                    start=(kb == 0),
                                    stop=(kb == kb_max - 1),
                                    skip_group_check=True)

                        for kb in range(kb_max):
                            o = max(0, 128 * kb - q0)
                            L = 512 - o
                            sps = psS.tile([128, 1024], F32, tag="sps",
                                           name=f"sps{si}_{p}_{kb}")
                            for hl in range(2):
                                hb = 64 * hl
                                nc.tensor.matmul(
                                    sps[:, 512 * hl + o:512 * hl + 512],
                                    kT[p][hb:hb + 64,
                                          kb * 128:(kb + 1) * 128],
                                    qT[p][hb:hb + 64, q0 + o:q0 + 512],
                                    start=True, stop=True)
                            ptb = ptp.tile([128, 1024], BF16, tag="ptb",
                                           name=f"ptb{si}_{p}_{kb}")
                            sps3 = sps[:].rearrange("a (h q) -> a h q", h=2)
                            ptb3 = ptb[:].rearrange("a (h q) -> a h q", h=2)
                            nc.scalar.activation(
                                ptb3[:, :, 0:L], sps3[:, :, o:512],
                                AF.Exp, scale=SCALE)
                            if o == 128 * kb - q0:
                                # diagonal block: causal mask on cols 0:128
                                # (keep col j iff j >= partition)
                                for hl in range(2):
                                    nc.gpsimd.affine_select(
                                        ptb[:, 512 * hl:512 * hl + 128],
                                        ptb[:, 512 * hl:512 * hl + 128],
                                        pattern=[[1, 128]],
                                        compare_op=ALU.is_ge, fill=0.0,
                                        base=0, channel_multiplier=-1)
                            pend.append((kb, o, ptb))
                            # keep one S block in flight ahead of AV
                            if len(pend) > 1:
                                emit_av(*pend.pop(0))
                        while pend:
                            emit_av(*pend.pop(0))
                        # division: row 64 of av is the denominator
                        r_sb = rp.tile([1, 1024], F32, tag="r",
                                       name=f"rsb{si}_{p}")
                        nc.vector.reciprocal(r_sb[:], av[64:65, :])
                        rb = rp.tile([64, 1024], F32, tag="rb",
                                     name=f"rbb{si}_{p}")
                        nc.gpsimd.partition_broadcast(rb[:], r_sb[:])
                        for hl in range(2):
                            nc.vector.tensor_tensor(
                                out=oT[p][64 * hl:64 * hl + 64,
                                          q0:q0 + 512],
                                in0=av[0:64, hl * 512:(hl + 1) * 512],
                                in1=rb[:, hl * 512:(hl + 1) * 512],
                                op=ALU.mult)

                    def emit_oproj(si):
                        q0 = 512 * si
                        for j in range(4):
                            pD = psS.tile([128, 1024], F32, tag="sps",
                                          name=f"pD{si}_{j}")
                            for nn in range(2):
                                n = 2 * j + nn
                                for p in range(2):
                                    nc.tensor.matmul(
                                        pD[:, nn * 512:(nn + 1) * 512],
                                        wo_sb[p][:, n * 128:(n + 1) * 128],
                                        oT[p][:, q0:q0 + 512],
                                        start=(p == 0), stop=(p == 1))
                            fo = fop.tile([128, 1024], BF16, tag="fo",
                                          name=f"fo{si}_{j}")
                            nc.vector.tensor_copy(fo[:], pD[:])
                            for nn in range(2):
                                n = 2 * j + nn
                                nc.sync.dma_start(
                                    out=d_out[n * 128:(n + 1) * 128,
                                              q0:q0 + 512],
                                    in_=fo[:, nn * 512:(nn + 1) * 512])

                    for si in range(4):
                        emit_strip(si, 0)
                        if si > 0:
                            emit_oproj(si - 1)
                        emit_strip(si, 1)
                    emit_oproj(3)

            psS.release()
            psV.release()

    nc.compile()
    return nc


_NC_CACHE = None


def _get_program():
    global _NC_CACHE
    if _NC_CACHE is None:
        _NC_CACHE = _build_program()
    return _NC_CACHE


def _rope_tables():
    inv_freq = 1.0 / (10000.0 ** (np.arange(0, HD, 2, dtype=np.float32) / HD))
    freqs = np.outer(np.arange(T, dtype=np.float32), inv_freq)  # [T, 32]
    emb = np.concatenate([freqs, freqs], axis=-1)               # [T, 64]
    return np.cos(emb), np.sin(emb)


def _to_bf16(a):
    import ml_dtypes
    return np.asarray(a, dtype=np.float32).astype(ml_dtypes.bfloat16)


def _host_prep(x, w_qkv, w_out):
    cos, sin = _rope_tables()          # [T, 64] each, original hd order
    # permuted + transposed tables [64, T], duplicated for a 2-head pair tile
    cosP = np.ascontiguousarray(cos.T[PI, :])                   # [64, T]
    sinP = sin.T[PI, :].copy()                                  # [64, T]
    sinP[0::2, :] *= -1.0                                       # sign baked in
    cos2 = _to_bf16(np.vstack([cosP, cosP]))
    sin2 = _to_bf16(np.vstack([sinP, sinP]))
    ident = _to_bf16(np.eye(128, dtype=np.float32))
    ones16 = _to_bf16(np.ones((128, 16), dtype=np.float32))

    in_maps = []
    for core in range(NCORES):
        b = core // GROUPS
        h0 = (core % GROUPS) * HPC
        xT = np.ascontiguousarray(x[b].T)                       # [D, T]
        cols = []
        for p in range(2):                                      # v (no perm)
            for hh in range(2):
                h = h0 + 2 * p + hh
                cols.append(w_qkv[:, 2 * D + h * HD:2 * D + (h + 1) * HD])
        for kind in range(2):                                   # q, k
            for p in range(2):                                  # head pairs
                for hh in range(2):
                    h = h0 + 2 * p + hh
                    wcol = w_qkv[:, kind * D + h * HD:kind * D + (h + 1) * HD]
                    cols.append(wcol[:, PI])
        w_cat = np.concatenate(cols, axis=1)                    # [D, 768]
        w_o = w_out[h0 * HD:(h0 + HPC) * HD, :]                 # [256, D]
        in_maps.append({
            "xT": _to_bf16(xT),
            "w_cat": _to_bf16(w_cat),
            "w_o": _to_bf16(w_o),
            "cos2": cos2,
            "sin2": sin2,
            "ident": ident,
            "ones16": ones16,
        })
    return in_maps


def kernel(x, w_qkv, w_out):
    x = np.asarray(x, dtype=np.float32)
    w_qkv = np.asarray(w_qkv, dtype=np.float32)
    w_out = np.asarray(w_out, dtype=np.float32)
    nc = _get_program()
    in_maps = _host_prep(x, w_qkv, w_out)
    trace = bool(int(os.environ.get("KBENCH_TRACE", "0")))
    res = run_bass_kernel_spmd(nc, in_maps, list(range(NCORES)), trace=trace)
    if trace and res.exec_time_ns is not None:
        print(f"HW exec time: {res.exec_time_ns} ns")
    out = np.zeros((B, T, D), dtype=np.float32)
    for core in range(NCORES):
        b = core // GROUPS
        out[b] += res.results[core]["outp"].T.astype(np.float32)
    return out


# revision 7
# speedup vs baseline: 1.1752x; 1.1752x over previous
# Causal self-attention (B=2, T=2048, D=1024, H=16, HD=64) with RoPE on 8 TRN2
# cores.
#
# Sharding: data-parallel over batch (2 groups of 4 cores), tensor-parallel
# over heads within each group (4 heads per core, as 2 head-pairs p=0,1).
# Everything on-device is bf16 (inputs pre-converted on host): bf16 matmuls run
# at 1 cycle/row at ANY moving size (no fp32r >=256 constraint), DVE
# elementwise ops get the 2x packed mode, and DMA bytes are halved.
#
# Per core:
#   Phase A - stream x by 512-col t-chunks; for each chunk accumulate the six
#     128-col qkv projections (q/k/v x 2 pairs) over 8 contraction tiles.
#     t0/t1 interleave the contraction tiles across all six outputs (DMA-
#     paced); t2/t3 run output-major with immediate per-output drains so the
#     PSUM banks hand over to attention without a bubble. Drains: RoPE (Act
#     copy + DVE shuffle/mul/mul/add) for q/k, PE transposes + Act copies
#     into the AV-stationary layout for v (ones column appended by memset ->
#     softmax denominator comes free out of the AV matmul).
#   Attention - per 512-row q strip and head pair: S^T blocks with the next
#     S emitted before the previous AV so the PE never waits on exp; exp on
#     the Scalar engine (fully-causal block pairs share one call per head to
#     amortize the PSUM-access surcharge), causal mask via affine_select on
#     diagonal blocks only, AV accumulating [65, q] (row 64 = denominator),
#     then reciprocal/partition-broadcast/mul into oT.
#   Out-projection - row-sharded partial [D, T]; interleaved one strip behind
#     attention reusing the S-psum tag (PSUM stays within 8 banks), drains
#     split across Pool and DVE so they never queue behind the division
#     chain. Host sums the 4 partials per batch and transposes back.
import sys
import os

sys.path.insert(0, "/opt/trn_rl_repo")

import numpy as np

import concourse.bass as bass  # noqa: F401  (bass types used via bacc)
import concourse.mybir as mybir
from concourse import bacc
from concourse.tile import TileContext
from concourse.bass_utils import run_bass_kernel_spmd
from contextlib import ExitStack

F32 = mybir.dt.float32
BF16 = mybir.dt.bfloat16
AF = mybir.ActivationFunctionType
ALU = mybir.AluOpType

B, T, D = 2, 2048, 1024
H, HD = 16, 64
NCORES = 8
GROUPS = NCORES // B          # cores per batch = 4
HPC = H // GROUPS             # heads per core = 4
NK = D // 128                 # contraction tiles for D
SCALE = HD ** -0.5

# hd interleave: new row 2j <- orig j, new row 2j+1 <- orig j+32 so the
# rotate-half partner of every row is its neighbour (swappable by a 32-lane
# stream shuffle).
PI = np.empty(HD, dtype=np.int64)
PI[0::2] = np.arange(32)
PI[1::2] = np.arange(32, 64)

SWAP_MASK = []
for _i in range(16):
    SWAP_MASK += [2 * _i + 1, 2 * _i]

# c-tile order: q0, k0 first so pair-0 RoPE drains earliest; v before pair-1
# so the AV stationaries are ready before attention begins.
C_ORDER = [2, 4, 0, 1, 3, 5]


def _build_program():
    nc = bacc.Bacc("TRN2", target_bir_lowering=False, debug=False,
                   num_devices=NCORES)
    d_x = nc.dram_tensor("xT", [D, T], BF16, kind="ExternalInput").ap()
    d_w = nc.dram_tensor("w_cat", [D, 6 * 128], BF16,
                         kind="ExternalInput").ap()
    d_wo = nc.dram_tensor("w_o", [2 * 128, D], BF16,
                          kind="ExternalInput").ap()
    d_cos = nc.dram_tensor("cos2", [128, T], BF16, kind="ExternalInput").ap()
    d_sin = nc.dram_tensor("sin2", [128, T], BF16, kind="ExternalInput").ap()
    d_id = nc.dram_tensor("ident", [128, 128], BF16,
                          kind="ExternalInput").ap()
    # [si*4+j, r, nn*512+q] blocks; host reassembles to [D, T]
    d_out = nc.dram_tensor("outp", [16, 128, 1024], BF16,
                           kind="ExternalOutput").ap()

    with TileContext(nc) as tc, nc.allow_low_precision(reason="bf16 attn"):
        with ExitStack() as root:
            qkv_pool = root.enter_context(tc.tile_pool(name="qkv", bufs=1))
            va_pool = root.enter_context(tc.tile_pool(name="va", bufs=1))
            out_pool = root.enter_context(tc.tile_pool(name="outT", bufs=1))
            wop = root.enter_context(tc.tile_pool(name="wop", bufs=1))
            wu_pool = root.enter_context(tc.tile_pool(name="wu", bufs=1))

            qT = [qkv_pool.tile([128, T], BF16, tag=f"q{p}", name=f"qT{p}")
                  for p in range(2)]
            kT = [qkv_pool.tile([128, T], BF16, tag=f"k{p}", name=f"kTt{p}")
                  for p in range(2)]
            va = [va_pool.tile([128, 16 * 65], BF16, tag=f"va{h}",
                               name=f"va{h}") for h in range(HPC)]
            oT = [out_pool.tile([128, T], BF16, tag=f"o{p}", name=f"oT{p}")
                  for p in range(2)]
            wo_sb = [wop.tile([128, D], BF16, tag=f"wo{p}", name=f"wo{p}")
                     for p in range(2)]

            # Warm the Act engine's exp table before it matters.
            wu = wu_pool.tile([1, 2], F32, tag="wu")
            wu2 = wu_pool.tile([1, 2], F32, tag="wu2")
            nc.vector.memset(wu[:], 0.0)
            nc.scalar.activation(wu2[:], wu[:], AF.Exp, scale=1.0)
            # denominator ones columns (free on Pool; avoids a strided DMA)
            for h in range(HPC):
                nc.gpsimd.memset(va[h][:, 64:16 * 65:65], 1.0)

            # ---------------- Phase A: qkv projection + RoPE + v transpose
            with nc.named_scope("qkv"):
                with ExitStack() as sA:
                    tab = sA.enter_context(tc.tile_pool(name="tab", bufs=1))
                    xp = sA.enter_context(tc.tile_pool(name="xp", bufs=1))
                    wp = sA.enter_context(tc.tile_pool(name="wp", bufs=1))
                    tp = sA.enter_context(tc.tile_pool(name="ropetmp",
                                                       bufs=2))
                    vtp = sA.enter_context(tc.tile_pool(name="vT", bufs=1))

                    cos2 = tab.tile([128, T], BF16, tag="cos")
                    sin2 = tab.tile([128, T], BF16, tag="sin")
                    ident = tab.tile([128, 128], BF16, tag="id")
                    vT = [vtp.tile([128, T], BF16, tag=f"v{p}",
                                   name=f"vT{p}") for p in range(2)]
                    x_sb = [xp.tile([128, T], BF16, tag=f"x{kt}",
                                    name=f"xsb{kt}") for kt in range(NK)]
                    w_sb = [wp.tile([128, 6 * 128], BF16, tag=f"w{kt}",
                                    name=f"wsb{kt}") for kt in range(NK)]

                    # DMA queue: x(kt,t0)+w(kt) pairs stream first so the PE
                    # starts ~1us in; tables follow in consumption order.
                    for kt in range(NK):
                        nc.sync.dma_start(out=x_sb[kt][:, 0:512],
                                          in_=d_x[kt * 128:(kt + 1) * 128,
                                                  0:512])
                        nc.sync.dma_start(out=w_sb[kt][:],
                                          in_=d_w[kt * 128:(kt + 1) * 128, :])
                        if kt == 0:
                            nc.sync.dma_start(out=cos2[:, 0:512],
                                              in_=d_cos[:, 0:512])
                            nc.sync.dma_start(out=sin2[:, 0:512],
                                              in_=d_sin[:, 0:512])
                    nc.sync.dma_start(out=cos2[:, 512:1024],
                                      in_=d_cos[:, 512:1024])
                    nc.sync.dma_start(out=sin2[:, 512:1024],
                                      in_=d_sin[:, 512:1024])
                    nc.sync.dma_start(out=ident[:], in_=d_id[:])
                    for kt in range(NK):
                        nc.sync.dma_start(
                            out=x_sb[kt][:, 512:2048],
                            in_=d_x[kt * 128:(kt + 1) * 128, 512:2048])
                    nc.sync.dma_start(out=cos2[:, 1024:2048],
                                      in_=d_cos[:, 1024:2048])
                    nc.sync.dma_start(out=sin2[:, 1024:2048],
                                      in_=d_sin[:, 1024:2048])
                    for p in range(2):
                        nc.sync.dma_start(
                            out=wo_sb[p][:],
                            in_=d_wo[p * 128:(p + 1) * 128, :])

                    accp = tc.alloc_tile_pool(name="accs", bufs=1,
                                              space="PSUM")
                    psT = tc.alloc_tile_pool(name="psT", bufs=2, space="PSUM",
                                             side="right")

                    def emit_rope(c, acc, tsl):
                        dst = qT[c - 2] if c in (2, 3) else kT[c - 4]
                        qsb = tp.tile([128, 512], BF16, tag="qsb")
                        nc.scalar.copy(qsb[:], acc[:])
                        qsh = tp.tile([128, 512], BF16, tag="qsh")
                        nc.vector.stream_shuffle(qsh[:], acc[:], SWAP_MASK)
                        tcos = tp.tile([128, 512], BF16, tag="tcos")
                        nc.vector.tensor_tensor(out=tcos[:], in0=qsb[:],
                                                in1=cos2[:, tsl],
                                                op=ALU.mult)
                        nc.vector.tensor_tensor(out=qsh[:], in0=qsh[:],
                                                in1=sin2[:, tsl],
                                                op=ALU.mult)
                        nc.vector.tensor_tensor(out=dst[:, tsl], in0=tcos[:],
                                                in1=qsh[:], op=ALU.add)

                    def emit_vtrans(t):
                        # transposes + va copies for both v c-tiles of chunk t
                        for p in range(2):
                            pt_ = psT.tile([128, 512], BF16, tag="pt",
                                           name=f"ptr{p}_{t}")
                            for j in range(4):
                                tt = 4 * t + j
                                nc.tensor.transpose(
                                    pt_[:, j * 128:(j + 1) * 128],
                                    vT[p][:, tt * 128:(tt + 1) * 128],
                                    ident[:])
                            for j in range(4):
                                tt = 4 * t + j
                                nc.scalar.copy(
                                    va[2 * p][:, tt * 65:tt * 65 + 64],
                                    pt_[:, j * 128:j * 128 + 64])
                                nc.scalar.copy(
                                    va[2 * p + 1][:, tt * 65:tt * 65 + 64],
                                    pt_[:, j * 128 + 64:j * 128 + 128])

                    def drain(c, acc, tsl):
                        if c in (0, 1):
                            nc.scalar.copy(vT[c][:, tsl], acc[:])
                        else:
                            emit_rope(c, acc, tsl)

                    for t in range(4):
                        tsl = slice(t * 512, (t + 1) * 512)
                        accs = {c: accp.tile([128, 512], F32, tag=f"a{c}",
                                             name=f"acc{c}_{t}")
                                for c in C_ORDER}
                        if t < 2:
                            # contraction-tile inner: matches the x DMA pace
                            for kt in range(NK):
                                for c in C_ORDER:
                                    nc.tensor.matmul(
                                        accs[c][:],
                                        w_sb[kt][:, c * 128:(c + 1) * 128],
                                        x_sb[kt][:, tsl],
                                        start=(kt == 0), stop=(kt == NK - 1))
                            if t == 1:
                                emit_vtrans(0)
                            for c in C_ORDER:
                                drain(c, accs[c], tsl)
                        else:
                            # output-major with immediate drains: PSUM banks
                            # free progressively, so attention starts with no
                            # bubble after t=3.
                            for ci, c in enumerate(C_ORDER):
                                for kt in range(NK):
                                    nc.tensor.matmul(
                                        accs[c][:],
                                        w_sb[kt][:, c * 128:(c + 1) * 128],
                                        x_sb[kt][:, tsl],
                                        start=(kt == 0), stop=(kt == NK - 1))
                                if ci == 5:
                                    emit_vtrans(t - 1)
                                drain(c, accs[c], tsl)
                    emit_vtrans(3)
                    psT.release()
                    accp.release()

            # ---------------- attention + interleaved out-projection
            psS = tc.alloc_tile_pool(name="psS", bufs=2, space="PSUM")
            psV = tc.alloc_tile_pool(name="psV", bufs=2, space="PSUM",
                                     side="right")

            with nc.named_scope("attn"):
                with ExitStack() as sB:
                    ptp = sB.enter_context(tc.tile_pool(name="ptp", bufs=6))
                    rp = sB.enter_context(tc.tile_pool(name="rp", bufs=2))
                    fop = sB.enter_context(tc.tile_pool(name="fop", bufs=3))

                    def emit_strip(si, p):
                        q0 = 512 * si
                        kb_max = 4 * (si + 1)
                        av = psV.tile([65, 1024], F32, tag="av",
                                      name=f"avps{si}_{p}")
                        # units: fully-causal kb pairs (one exp per head),
                        # then the 4 diagonal blocks individually
                        units = [("pair", kb) for kb in range(0, 4 * si, 2)]
                        units += [("diag", kb) for kb in range(4 * si,
                                                               kb_max)]
                        pend = []

                        def emit_s(unit):
                            kind, kb = unit
                            if kind == "pair":
                                ptbs = []
                                for hl in range(2):
                                    hb = 64 * hl
                                    sps = psS.tile(
                                        [128, 1024], F32, tag="sps",
                                        name=f"sp{si}_{p}_{kb}_{hl}")
                                    for dk in range(2):
                                        nc.tensor.matmul(
                                            sps[:, dk * 512:(dk + 1) * 512],
                                            kT[p][hb:hb + 64,
                                                  (kb + dk) * 128:
                                                  (kb + dk + 1) * 128],
                                            qT[p][hb:hb + 64, q0:q0 + 512],
                                            start=True, stop=True)
                                    ptb = ptp.tile(
                                        [128, 1024], BF16, tag="ptb",
                                        name=f"pt{si}_{p}_{kb}_{hl}")
                                    nc.scalar.activation(
                                        ptb[:], sps[:], AF.Exp, scale=SCALE)
                                    ptbs.append(ptb)
                                return ptbs
                            # diagonal block: both heads in one sps tile
                            o = 128 * kb - q0
                            L = 512 - o
                            sps = psS.tile([128, 1024], F32, tag="sps",
                                           name=f"sp{si}_{p}_{kb}")
                            for hl in range(2):
                                hb = 64 * hl
                                nc.tensor.matmul(
                                    sps[:, 512 * hl + o:512 * hl + 512],
                                    kT[p][hb:hb + 64,
                                          kb * 128:(kb + 1) * 128],
                                    qT[p][hb:hb + 64, q0 + o:q0 + 512],
                                    start=True, stop=True)
                            ptb = ptp.tile([128, 1024], BF16, tag="ptb",
                                           name=f"pt{si}_{p}_{kb}")
                            sps3 = sps[:].rearrange("a (h q) -> a h q", h=2)
                            ptb3 = ptb[:].rearrange("a (h q) -> a h q", h=2)
                            nc.scalar.activation(
                                ptb3[:, :, 0:L], sps3[:, :, o:512],
                                AF.Exp, scale=SCALE)
                            # causal mask: keep col j iff j >= partition
                            for hl in range(2):
                                nc.gpsimd.affine_select(
                                    ptb[:, 512 * hl:512 * hl + 128],
                                    ptb[:, 512 * hl:512 * hl + 128],
                                    pattern=[[1, 128]],
                                    compare_op=ALU.is_ge, fill=0.0,
                                    base=0, channel_multiplier=-1)
                            return ptb

                        def emit_av(unit, ptbs):
                            kind, kb = unit
                            if kind == "pair":
                                for hl in range(2):
                                    for dk in range(2):
                                        nc.tensor.matmul(
                                            av[:, hl * 512:(hl + 1) * 512],
                                            va[2 * p + hl][
                                                :, (kb + dk) * 65:
                                                (kb + dk) * 65 + 65],
                                            ptbs[hl][:, dk * 512:
                                                     (dk + 1) * 512],
                                            start=(kb + dk == 0),
                                            stop=(kb + dk == kb_max - 1),
                                            skip_group_check=True)
                            else:
                                o = 128 * kb - q0
                                L = 512 - o
                                for hl in range(2):
                                    nc.tensor.matmul(
                                        av[:, hl * 512 + o:hl * 512 + 512],
                                        va[2 * p + hl][:, kb * 65:
                                                       kb * 65 + 65],
                                        ptbs[:, 512 * hl:512 * hl + L],
                                        start=(kb == 0),
                                        stop=(kb == kb_max - 1),
                                        skip_group_check=True)

                        for unit in units:
                            pend.append((unit, emit_s(unit)))
                            if len(pend) > 1:
                                emit_av(*pend.pop(0))
                        while pend:
                            emit_av(*pend.pop(0))
                        # division: row 64 of av is the denominator
                        r_sb = rp.tile([1, 1024], F32, tag="r",
                                       name=f"rsb{si}_{p}")
                        nc.vector.reciprocal(r_sb[:], av[64:65, :])
                        rb = rp.tile([64, 1024], F32, tag="rb",
                                     name=f"rbb{si}_{p}")
                        nc.gpsimd.partition_broadcast(rb[:], r_sb[:])
                        for hl in range(2):
                            nc.vector.tensor_tensor(
                                out=oT[p][64 * hl:64 * hl + 64,
                                          q0:q0 + 512],
                                in0=av[0:64, hl * 512:(hl + 1) * 512],
                                in1=rb[:, hl * 512:(hl + 1) * 512],
                                op=ALU.mult)

                    def emit_oproj(si):
                        q0 = 512 * si
                        for j in range(4):
                            pD = psS.tile([128, 1024], F32, tag="sps",
                                          name=f"pD{si}_{j}")
                            for nn in range(2):
                                n = 2 * j + nn
                                for p in range(2):
                                    nc.tensor.matmul(
                                        pD[:, nn * 512:(nn + 1) * 512],
                                        wo_sb[p][:, n * 128:(n + 1) * 128],
                                        oT[p][:, q0:q0 + 512],
                                        start=(p == 0), stop=(p == 1))
                            fo = fop.tile([128, 1024], BF16, tag="fo",
                                          name=f"fo{si}_{j}")
                            # drains split Pool/DVE so they never sit behind
                            # the division chain on one queue
                            if j % 2 == 0:
                                nc.gpsimd.tensor_copy(fo[:], pD[:])
                            else:
                                nc.vector.tensor_copy(fo[:], pD[:])
                            nc.sync.dma_start(out=d_out[4 * si + j],
                                              in_=fo[:])

                    for si in range(4):
                        emit_strip(si, 0)
                        if si > 0:
                            emit_oproj(si - 1)
                        emit_strip(si, 1)
                    emit_oproj(3)

            psS.release()
            psV.release()

    nc.compile()
    return nc


_NC_CACHE = None


def _get_program():
    global _NC_CACHE
    if _NC_CACHE is None:
        _NC_CACHE = _build_program()
    return _NC_CACHE


def _rope_tables():
    inv_freq = 1.0 / (10000.0 ** (np.arange(0, HD, 2, dtype=np.float32) / HD))
    freqs = np.outer(np.arange(T, dtype=np.float32), inv_freq)  # [T, 32]
    emb = np.concatenate([freqs, freqs], axis=-1)               # [T, 64]
    return np.cos(emb), np.sin(emb)


def _to_bf16(a):
    import ml_dtypes
    return np.asarray(a, dtype=np.float32).astype(ml_dtypes.bfloat16)


def _host_prep(x, w_qkv, w_out):
    cos, sin = _rope_tables()          # [T, 64] each, original hd order
    # permuted + transposed tables [64, T], duplicated for a 2-head pair tile
    cosP = np.ascontiguousarray(cos.T[PI, :])                   # [64, T]
    sinP = sin.T[PI, :].copy()                                  # [64, T]
    sinP[0::2, :] *= -1.0                                       # sign baked in
    cos2 = _to_bf16(np.vstack([cosP, cosP]))
    sin2 = _to_bf16(np.vstack([sinP, sinP]))
    ident = _to_bf16(np.eye(128, dtype=np.float32))

    in_maps = []
    for core in range(NCORES):
        b = core // GROUPS
        h0 = (core % GROUPS) * HPC
        xT = np.ascontiguousarray(x[b].T)                       # [D, T]
        cols = []
        for p in range(2):                                      # v (no perm)
            for hh in range(2):
                h = h0 + 2 * p + hh
                cols.append(w_qkv[:, 2 * D + h * HD:2 * D + (h + 1) * HD])
        for kind in range(2):                                   # q, k
            for p in range(2):                                  # head pairs
                for hh in range(2):
                    h = h0 + 2 * p + hh
                    wcol = w_qkv[:, kind * D + h * HD:kind * D + (h + 1) * HD]
                    cols.append(wcol[:, PI])
        w_cat = np.concatenate(cols, axis=1)                    # [D, 768]
        w_o = w_out[h0 * HD:(h0 + HPC) * HD, :]                 # [256, D]
        in_maps.append({
            "xT": _to_bf16(xT),
            "w_cat": _to_bf16(w_cat),
            "w_o": _to_bf16(w_o),
            "cos2": cos2,
            "sin2": sin2,
            "ident": ident,
        })
    return in_maps


def kernel(x, w_qkv, w_out):
    x = np.asarray(x, dtype=np.float32)
    w_qkv = np.asarray(w_qkv, dtype=np.float32)
    w_out = np.asarray(w_out, dtype=np.float32)
    nc = _get_program()
    in_maps = _host_prep(x, w_qkv, w_out)
    trace = bool(int(os.environ.get("KBENCH_TRACE", "0")))
    res = run_bass_kernel_spmd(nc, in_maps, list(range(NCORES)), trace=trace)
    if trace and res.exec_time_ns is not None:
        print(f"HW exec time: {res.exec_time_ns} ns")
    out = np.zeros((B, T, D), dtype=np.float32)
    for core in range(NCORES):
        b = core // GROUPS
        blk = res.results[core]["outp"].astype(np.float32)
        # (si, j, r, nn, q) -> rows (j, nn, r) = D, cols (si, q) = T
        dT = blk.reshape(4, 4, 128, 2, 512).transpose(1, 3, 2, 0, 4)
        out[b] += dT.reshape(D, T).T
    return out


# revision 8
# speedup vs baseline: 1.2376x; 1.0531x over previous
# Causal self-attention (B=2, T=2048, D=1024, H=16, HD=64) with RoPE on 8 TRN2
# cores.
#
# Sharding: data-parallel over batch (2 groups of 4 cores), tensor-parallel
# over heads within each group (4 heads per core, as 2 head-pairs p=0,1).
# Everything on-device is bf16 (inputs pre-converted on host): bf16 matmuls run
# at 1 cycle/row at ANY moving size (no fp32r >=256 constraint), DVE
# elementwise ops get the 2x packed mode, and DMA bytes are halved.
#
# Per core:
#   Phase A - stream x by 512-col t-chunks; for each chunk accumulate the six
#     128-col qkv projections (q/k/v x 2 pairs) over 8 contraction tiles.
#     t0/t1 interleave the contraction tiles across all six outputs (DMA-
#     paced); t2/t3 run output-major with immediate per-output drains so the
#     PSUM banks hand over to attention without a bubble. Drains: RoPE (Pool
#     copy + DVE shuffle/mul/mul/add) for q/k, PE transposes + Pool copies
#     into the AV-stationary layout for v (ones column appended by memset ->
#     softmax denominator comes free out of the AV matmul). The Act engine is
#     kept almost idle in phase A so it enters attention with no backlog.
#   Attention - per 512-row q strip and head pair: S^T blocks with the next
#     S emitted before the previous AV so the PE never waits on exp; exp on
#     the Scalar engine (the only engine with transcendentals - it is the
#     attention-phase bottleneck at ~73us so everything else stays off it),
#     causal mask via affine_select on diagonal blocks only, AV accumulating
#     [65, q] (row 64 = denominator), then per-head reciprocal/partition-
#     broadcast/mul into oT (split per head to halve the chain latency).
#   Out-projection - row-sharded partial [D, T]; interleaved one strip behind
#     attention on a dedicated PSUM tag, drains split across Pool and DVE so
#     they never queue behind the division chain. Host sums the 4 partials
#     per batch and transposes back.
import sys
import os

sys.path.insert(0, "/opt/trn_rl_repo")

import numpy as np

import concourse.bass as bass  # noqa: F401  (bass types used via bacc)
import concourse.mybir as mybir
from concourse import bacc
from concourse.tile import TileContext
from concourse.bass_utils import run_bass_kernel_spmd
from contextlib import ExitStack

F32 = mybir.dt.float32
BF16 = mybir.dt.bfloat16
AF = mybir.ActivationFunctionType
ALU = mybir.AluOpType

B, T, D = 2, 2048, 1024
H, HD = 16, 64
NCORES = 8
GROUPS = NCORES // B          # cores per batch = 4
HPC = H // GROUPS             # heads per core = 4
NK = D // 128                 # contraction tiles for D
SCALE = HD ** -0.5

# hd interleave: new row 2j <- orig j, new row 2j+1 <- orig j+32 so the
# rotate-half partner of every row is its neighbour (swappable by a 32-lane
# stream shuffle).
PI = np.empty(HD, dtype=np.int64)
PI[0::2] = np.arange(32)
PI[1::2] = np.arange(32, 64)

SWAP_MASK = []
for _i in range(16):
    SWAP_MASK += [2 * _i + 1, 2 * _i]

# w_cat column tiles (host order): c0=q pair0, c1=k pair0 (first so pair-0
# attention inputs drain earliest, and the first w DMA can cover just c0/c1),
# c2/c3 = v pairs, c4=q pair1, c5=k pair1.
ROPE_DST = {0: ("q", 0), 1: ("k", 0), 4: ("q", 1), 5: ("k", 1)}


def _build_program():
    nc = bacc.Bacc("TRN2", target_bir_lowering=False, debug=False,
                   num_devices=NCORES)
    d_x = nc.dram_tensor("xT", [D, T], BF16, kind="ExternalInput").ap()
    d_w = nc.dram_tensor("w_cat", [D, 6 * 128], BF16,
                         kind="ExternalInput").ap()
    d_wo = nc.dram_tensor("w_o", [2 * 128, D], BF16,
                          kind="ExternalInput").ap()
    d_cos = nc.dram_tensor("cos2", [128, T], BF16, kind="ExternalInput").ap()
    d_sin = nc.dram_tensor("sin2", [128, T], BF16, kind="ExternalInput").ap()
    d_id = nc.dram_tensor("ident", [128, 128], BF16,
                          kind="ExternalInput").ap()
    # [si*8+n, r, q] blocks; host reassembles to [D, T]
    d_out = nc.dram_tensor("outp", [32, 128, 512], BF16,
                           kind="ExternalOutput").ap()

    with TileContext(nc) as tc, nc.allow_low_precision(reason="bf16 attn"):
        with ExitStack() as root:
            qkv_pool = root.enter_context(tc.tile_pool(name="qkv", bufs=1))
            va_pool = root.enter_context(tc.tile_pool(name="va", bufs=1))
            out_pool = root.enter_context(tc.tile_pool(name="outT", bufs=1))
            wop = root.enter_context(tc.tile_pool(name="wop", bufs=1))
            wu_pool = root.enter_context(tc.tile_pool(name="wu", bufs=1))

            qT = [qkv_pool.tile([128, T], BF16, tag=f"q{p}", name=f"qT{p}")
                  for p in range(2)]
            kT = [qkv_pool.tile([128, T], BF16, tag=f"k{p}", name=f"kTt{p}")
                  for p in range(2)]
            va = [va_pool.tile([128, 16 * 65], BF16, tag=f"va{h}",
                               name=f"va{h}") for h in range(HPC)]
            oT = [out_pool.tile([128, T], BF16, tag=f"o{p}", name=f"oT{p}")
                  for p in range(2)]
            wo_sb = [wop.tile([128, D], BF16, tag=f"wo{p}", name=f"wo{p}")
                     for p in range(2)]

            # Warm the Act engine's exp table before it matters.
            wu = wu_pool.tile([1, 2], F32, tag="wu")
            wu2 = wu_pool.tile([1, 2], F32, tag="wu2")
            nc.vector.memset(wu[:], 0.0)
            nc.scalar.activation(wu2[:], wu[:], AF.Exp, scale=1.0)
            # denominator ones columns (free on Pool; avoids a strided DMA)
            for h in range(HPC):
                nc.gpsimd.memset(va[h][:, 64:16 * 65:65], 1.0)

            # ---------------- Phase A: qkv projection + RoPE + v transpose
            with nc.named_scope("qkv"):
                with ExitStack() as sA:
                    tab = sA.enter_context(tc.tile_pool(name="tab", bufs=1))
                    xp = sA.enter_context(tc.tile_pool(name="xp", bufs=1))
                    wp = sA.enter_context(tc.tile_pool(name="wp", bufs=1))
                    tp = sA.enter_context(tc.tile_pool(name="ropetmp",
                                                       bufs=2))
                    vtp = sA.enter_context(tc.tile_pool(name="vT", bufs=1))

                    cos2 = tab.tile([128, T], BF16, tag="cos")
                    sin2 = tab.tile([128, T], BF16, tag="sin")
                    ident = tab.tile([128, 128], BF16, tag="id")
                    vT = [vtp.tile([128, T], BF16, tag=f"v{p}",
                                   name=f"vT{p}") for p in range(2)]
                    x_sb = [xp.tile([128, T], BF16, tag=f"x{kt}",
                                    name=f"xsb{kt}") for kt in range(NK)]
                    w_sb = [wp.tile([128, 6 * 128], BF16, tag=f"w{kt}",
                                    name=f"wsb{kt}") for kt in range(NK)]

                    # DMA queue: x(kt,t0) + w(kt) pairs stream first (the
                    # first w transfer covers only q0/k0 so the PE starts
                    # ~2us in); tables follow in consumption order.
                    for kt in range(NK):
                        nc.sync.dma_start(out=x_sb[kt][:, 0:512],
                                          in_=d_x[kt * 128:(kt + 1) * 128,
                                                  0:512])
                        if kt == 0:
                            nc.sync.dma_start(out=w_sb[0][:, 0:256],
                                              in_=d_w[0:128, 0:256])
                            nc.sync.dma_start(out=w_sb[0][:, 256:768],
                                              in_=d_w[0:128, 256:768])
                            nc.sync.dma_start(out=cos2[:, 0:512],
                                              in_=d_cos[:, 0:512])
                            nc.sync.dma_start(out=sin2[:, 0:512],
                                              in_=d_sin[:, 0:512])
                        else:
                            nc.sync.dma_start(
                                out=w_sb[kt][:],
                                in_=d_w[kt * 128:(kt + 1) * 128, :])
                    nc.sync.dma_start(out=cos2[:, 512:1024],
                                      in_=d_cos[:, 512:1024])
                    nc.sync.dma_start(out=sin2[:, 512:1024],
                                      in_=d_sin[:, 512:1024])
                    nc.sync.dma_start(out=ident[:], in_=d_id[:])
                    for kt in range(NK):
                        nc.sync.dma_start(
                            out=x_sb[kt][:, 512:2048],
                            in_=d_x[kt * 128:(kt + 1) * 128, 512:2048])
                    nc.sync.dma_start(out=cos2[:, 1024:2048],
                                      in_=d_cos[:, 1024:2048])
                    nc.sync.dma_start(out=sin2[:, 1024:2048],
                                      in_=d_sin[:, 1024:2048])
                    for p in range(2):
                        nc.sync.dma_start(
                            out=wo_sb[p][:],
                            in_=d_wo[p * 128:(p + 1) * 128, :])

                    accp = tc.alloc_tile_pool(name="accs", bufs=1,
                                              space="PSUM")
                    psT = tc.alloc_tile_pool(name="psT", bufs=2, space="PSUM",
                                             side="right")

                    def emit_rope(c, acc, tsl):
                        kind, pair = ROPE_DST[c]
                        dst = qT[pair] if kind == "q" else kT[pair]
                        qsb = tp.tile([128, 512], BF16, tag="qsb")
                        nc.gpsimd.tensor_copy(qsb[:], acc[:])
                        qsh = tp.tile([128, 512], BF16, tag="qsh")
                        nc.vector.stream_shuffle(qsh[:], acc[:], SWAP_MASK)
                        tcos = tp.tile([128, 512], BF16, tag="tcos")
                        nc.vector.tensor_tensor(out=tcos[:], in0=qsb[:],
                                                in1=cos2[:, tsl],
                                                op=ALU.mult)
                        nc.vector.tensor_tensor(out=qsh[:], in0=qsh[:],
                                                in1=sin2[:, tsl],
                                                op=ALU.mult)
                        nc.vector.tensor_tensor(out=dst[:, tsl], in0=tcos[:],
                                                in1=qsh[:], op=ALU.add)

                    def emit_vtrans(t):
                        # transposes + va copies for both v c-tiles of chunk t
                        for p in range(2):
                            pt_ = psT.tile([128, 512], BF16, tag="pt",
                                           name=f"ptr{p}_{t}")
                            for j in range(4):
                                tt = 4 * t + j
                                nc.tensor.transpose(
                                    pt_[:, j * 128:(j + 1) * 128],
                                    vT[p][:, tt * 128:(tt + 1) * 128],
                                    ident[:])
                            for j in range(4):
                                tt = 4 * t + j
                                nc.gpsimd.tensor_copy(
                                    va[2 * p][:, tt * 65:tt * 65 + 64],
                                    pt_[:, j * 128:j * 128 + 64])
                                nc.gpsimd.tensor_copy(
                                    va[2 * p + 1][:, tt * 65:tt * 65 + 64],
                                    pt_[:, j * 128 + 64:j * 128 + 128])

                    def drain(c, acc, tsl):
                        if c in (2, 3):
                            nc.scalar.copy(vT[c - 2][:, tsl], acc[:])
                        else:
                            emit_rope(c, acc, tsl)

                    for t in range(4):
                        tsl = slice(t * 512, (t + 1) * 512)
                        accs = [accp.tile([128, 512], F32, tag=f"a{c}",
                                          name=f"acc{c}_{t}")
                                for c in range(6)]
                        if t < 2:
                            # contraction-tile inner: matches the x DMA pace
                            for kt in range(NK):
                                for c in range(6):
                                    nc.tensor.matmul(
                                        accs[c][:],
                                        w_sb[kt][:, c * 128:(c + 1) * 128],
                                        x_sb[kt][:, tsl],
                                        start=(kt == 0), stop=(kt == NK - 1))
                            if t == 1:
                                emit_vtrans(0)
                            for c in range(6):
                                drain(c, accs[c], tsl)
                        else:
                            # output-major with immediate drains: PSUM banks
                            # free progressively, so attention starts with no
                            # bubble after t=3.
                            for c in range(6):
                                for kt in range(NK):
                                    nc.tensor.matmul(
                                        accs[c][:],
                                        w_sb[kt][:, c * 128:(c + 1) * 128],
                                        x_sb[kt][:, tsl],
                                        start=(kt == 0), stop=(kt == NK - 1))
                                if c == 5:
                                    emit_vtrans(t - 1)
                                drain(c, accs[c], tsl)
                    emit_vtrans(3)
                    psT.release()
                    accp.release()

            # ---------------- attention + interleaved out-projection
            psS = tc.alloc_tile_pool(name="psS", bufs=2, space="PSUM")
            psV = tc.alloc_tile_pool(name="psV", bufs=1, space="PSUM",
                                     side="right")

            with nc.named_scope("attn"):
                with ExitStack() as sB:
                    ptp = sB.enter_context(tc.tile_pool(name="ptp", bufs=6))
                    rp = sB.enter_context(tc.tile_pool(name="rp", bufs=2))
                    fop = sB.enter_context(tc.tile_pool(name="fop", bufs=4))

                    def emit_strip(si, p):
                        q0 = 512 * si
                        kb_max = 4 * (si + 1)
                        av = psV.tile([65, 1024], F32, tag="av", bufs=1,
                                      name=f"avps{si}_{p}")
                        # units: fully-causal kb pairs (one exp per head),
                        # then the 4 diagonal blocks individually
                        units = [("pair", kb) for kb in range(0, 4 * si, 2)]
                        units += [("diag", kb) for kb in range(4 * si,
                                                               kb_max)]
                        pend = []

                        def emit_s(unit):
                            kind, kb = unit
                            if kind == "pair":
                                ptbs = []
                                for hl in range(2):
                                    hb = 64 * hl
                                    sps = psS.tile(
                                        [128, 1024], F32, tag="sps",
                                        name=f"sp{si}_{p}_{kb}_{hl}")
                                    for dk in range(2):
                                        nc.tensor.matmul(
                                            sps[:, dk * 512:(dk + 1) * 512],
                                            kT[p][hb:hb + 64,
                                                  (kb + dk) * 128:
                                                  (kb + dk + 1) * 128],
                                            qT[p][hb:hb + 64, q0:q0 + 512],
                                            start=True, stop=True)
                                    ptb = ptp.tile(
                                        [128, 1024], BF16, tag="ptb",
                                        name=f"pt{si}_{p}_{kb}_{hl}")
                                    nc.scalar.activation(
                                        ptb[:], sps[:], AF.Exp, scale=SCALE)
                                    ptbs.append(ptb)
                                return ptbs
                            # diagonal block: both heads in one sps tile
                            o = 128 * kb - q0
                            L = 512 - o
                            sps = psS.tile([128, 1024], F32, tag="sps",
                                           name=f"sp{si}_{p}_{kb}")
                            for hl in range(2):
                                hb = 64 * hl
                                nc.tensor.matmul(
                                    sps[:, 512 * hl + o:512 * hl + 512],
                                    kT[p][hb:hb + 64,
                                          kb * 128:(kb + 1) * 128],
                                    qT[p][hb:hb + 64, q0 + o:q0 + 512],
                                    start=True, stop=True)
                            ptb = ptp.tile([128, 1024], BF16, tag="ptb",
                                           name=f"pt{si}_{p}_{kb}")
                            sps3 = sps[:].rearrange("a (h q) -> a h q", h=2)
                            ptb3 = ptb[:].rearrange("a (h q) -> a h q", h=2)
                            nc.scalar.activation(
                                ptb3[:, :, 0:L], sps3[:, :, o:512],
                                AF.Exp, scale=SCALE)
                            # causal mask: keep col j iff j >= partition
                            for hl in range(2):
                                nc.gpsimd.affine_select(
                                    ptb[:, 512 * hl:512 * hl + 128],
                                    ptb[:, 512 * hl:512 * hl + 128],
                                    pattern=[[1, 128]],
                                    compare_op=ALU.is_ge, fill=0.0,
                                    base=0, channel_multiplier=-1)
                            return ptb

                        def emit_av(unit, ptbs):
                            kind, kb = unit
                            if kind == "pair":
                                for hl in range(2):
                                    for dk in range(2):
                                        nc.tensor.matmul(
                                            av[:, hl * 512:(hl + 1) * 512],
                                            va[2 * p + hl][
                                                :, (kb + dk) * 65:
                                                (kb + dk) * 65 + 65],
                                            ptbs[hl][:, dk * 512:
                                                     (dk + 1) * 512],
                                            start=(kb + dk == 0),
                                            stop=(kb + dk == kb_max - 1),
                                            skip_group_check=True)
                            else:
                                o = 128 * kb - q0
                                L = 512 - o
                                for hl in range(2):
                                    nc.tensor.matmul(
                                        av[:, hl * 512 + o:hl * 512 + 512],
                                        va[2 * p + hl][:, kb * 65:
                                                       kb * 65 + 65],
                                        ptbs[:, 512 * hl:512 * hl + L],
                                        start=(kb == 0),
                                        stop=(kb == kb_max - 1),
                                        skip_group_check=True)

                        for unit in units:
                            pend.append((unit, emit_s(unit)))
                            if len(pend) > 1:
                                emit_av(*pend.pop(0))
                        while pend:
                            emit_av(*pend.pop(0))
                        # division, split per head to halve chain latency:
                        # row 64 of av is the denominator
                        for hl in range(2):
                            hsl = slice(hl * 512, (hl + 1) * 512)
                            r_sb = rp.tile([1, 512], F32, tag="r",
                                           name=f"rsb{si}_{p}_{hl}")
                            nc.vector.reciprocal(r_sb[:], av[64:65, hsl])
                            rb = rp.tile([64, 512], F32, tag="rb",
                                         name=f"rbb{si}_{p}_{hl}")
                            nc.gpsimd.partition_broadcast(rb[:], r_sb[:])
                            nc.vector.tensor_tensor(
                                out=oT[p][64 * hl:64 * hl + 64,
                                          q0:q0 + 512],
                                in0=av[0:64, hsl], in1=rb[:],
                                op=ALU.mult)

                    def emit_oproj(si):
                        q0 = 512 * si
                        for n in range(8):
                            pD = psV.tile([128, 512], F32, tag="pd", bufs=2,
                                          name=f"pD{si}_{n}")
                            for p in range(2):
                                nc.tensor.matmul(
                                    pD[:],
                                    wo_sb[p][:, n * 128:(n + 1) * 128],
                                    oT[p][:, q0:q0 + 512],
                                    start=(p == 0), stop=(p == 1))
                            fo = fop.tile([128, 512], BF16, tag="fo",
                                          name=f"fo{si}_{n}")
                            # drains split Pool/DVE so they never sit behind
                            # the division chain on one queue
                            if n % 2 == 0:
                                nc.gpsimd.tensor_copy(fo[:], pD[:])
                            else:
                                nc.vector.tensor_copy(fo[:], pD[:])
                            nc.sync.dma_start(out=d_out[8 * si + n],
                                              in_=fo[:])

                    for si in range(4):
                        emit_strip(si, 0)
                        if si > 0:
                            emit_oproj(si - 1)
                        emit_strip(si, 1)
                    emit_oproj(3)

            psS.release()
            psV.release()

    nc.compile()
    return nc


_NC_CACHE = None


def _get_program():
    global _NC_CACHE
    if _NC_CACHE is None:
        _NC_CACHE = _build_program()
    return _NC_CACHE


def _rope_tables():
    inv_freq = 1.0 / (10000.0 ** (np.arange(0, HD, 2, dtype=np.float32) / HD))
    freqs = np.outer(np.arange(T, dtype=np.float32), inv_freq)  # [T, 32]
    emb = np.concatenate([freqs, freqs], axis=-1)               # [T, 64]
    return np.cos(emb), np.sin(emb)


def _to_bf16(a):
    import ml_dtypes
    return np.asarray(a, dtype=np.float32).astype(ml_dtypes.bfloat16)


def _host_prep(x, w_qkv, w_out):
    cos, sin = _rope_tables()          # [T, 64] each, original hd order
    # permuted + transposed tables [64, T], duplicated for a 2-head pair tile
    cosP = np.ascontiguousarray(cos.T[PI, :])                   # [64, T]
    sinP = sin.T[PI, :].copy()                                  # [64, T]
    sinP[0::2, :] *= -1.0                                       # sign baked in
    cos2 = _to_bf16(np.vstack([cosP, cosP]))
    sin2 = _to_bf16(np.vstack([sinP, sinP]))
    ident = _to_bf16(np.eye(128, dtype=np.float32))

    in_maps = []
    for core in range(NCORES):
        b = core // GROUPS
        h0 = (core % GROUPS) * HPC
        xT = np.ascontiguousarray(x[b].T)                       # [D, T]
        # column order: q pair0, k pair0, v pair0, v pair1, q pair1, k pair1
        cols = []
        for kind, p in [(0, 0), (1, 0)]:                        # q0, k0
            for hh in range(2):
                h = h0 + 2 * p + hh
                wcol = w_qkv[:, kind * D + h * HD:kind * D + (h + 1) * HD]
                cols.append(wcol[:, PI])
        for p in range(2):                                      # v (no perm)
            for hh in range(2):
                h = h0 + 2 * p + hh
                cols.append(w_qkv[:, 2 * D + h * HD:2 * D + (h + 1) * HD])
        for kind, p in [(0, 1), (1, 1)]:                        # q1, k1
            for hh in range(2):
                h = h0 + 2 * p + hh
                wcol = w_qkv[:, kind * D + h * HD:kind * D + (h + 1) * HD]
                cols.append(wcol[:, PI])
        w_cat = np.concatenate(cols, axis=1)                    # [D, 768]
        w_o = w_out[h0 * HD:(h0 + HPC) * HD, :]                 # [256, D]
        in_maps.append({
            "xT": _to_bf16(xT),
            "w_cat": _to_bf16(w_cat),
            "w_o": _to_bf16(w_o),
            "cos2": cos2,
            "sin2": sin2,
            "ident": ident,
        })
    return in_maps


def kernel(x, w_qkv, w_out):
    x = np.asarray(x, dtype=np.float32)
    w_qkv = np.asarray(w_qkv, dtype=np.float32)
    w_out = np.asarray(w_out, dtype=np.float32)
    nc = _get_program()
    in_maps = _host_prep(x, w_qkv, w_out)
    trace = bool(int(os.environ.get("KBENCH_TRACE", "0")))
    res = run_bass_kernel_spmd(nc, in_maps, list(range(NCORES)), trace=trace)
    if trace and res.exec_time_ns is not None:
        print(f"HW exec time: {res.exec_time_ns} ns")
    out = np.zeros((B, T, D), dtype=np.float32)
    for core in range(NCORES):
        b = core // GROUPS
        blk = res.results[core]["outp"].astype(np.float32)
        # (si, n, r, q) -> rows n*128+r = D, cols si*512+q = T
        dT = blk.reshape(4, 8, 128, 512).transpose(1, 2, 0, 3)
        out[b] += dT.reshape(D, T).T
    return out


# revision 15
# speedup vs baseline: 1.2799x; 1.0341x over previous
# Causal self-attention (B=2, T=2048, D=1024, H=16, HD=64) with RoPE on 8 TRN2
# cores.
#
# Sharding: data-parallel over batch (2 groups of 4 cores), tensor-parallel
# over heads within each group (4 heads per core, as 2 head-pairs p=0,1).
# Everything on-device is bf16 (inputs pre-converted on host): bf16 matmuls run
# at 1 cycle/row at ANY moving size (no fp32r >=256 constraint), DVE
# elementwise ops get the 2x packed mode, and DMA bytes are halved.
#
# Per core:
#   Phase A - stream x by 512-col t-chunks; for each chunk accumulate the six
#     128-col qkv projections (q/k/v x 2 pairs) over 8 contraction tiles.
#     t0/t1 interleave the contraction tiles across all six outputs (DMA-
#     paced); t2/t3 run output-major with immediate per-output drains so the
#     PSUM banks hand over to attention without a bubble. Drains: RoPE (Pool
#     copy + DVE shuffle/mul/mul/add) for q/k, PE transposes + Pool copies
#     into the AV-stationary layout for v (ones column appended by memset ->
#     softmax denominator comes free out of the AV matmul). The Act engine is
#     kept almost idle in phase A so it enters attention with no backlog.
#   Attention - per 512-row q strip and head pair: S^T blocks with the next
#     S emitted before the previous AV so the PE never waits on exp; exp on
#     the Scalar engine (the only engine with transcendentals - it is the
#     attention-phase bottleneck at ~73us so everything else stays off it),
#     causal mask via affine_select on diagonal blocks only, AV accumulating
#     [65, q] (row 64 = denominator), then per-head reciprocal/partition-
#     broadcast/mul into oT (split per head to halve the chain latency).
#   Out-projection - row-sharded partial [D, T]; interleaved one strip behind
#     attention on a dedicated PSUM tag, drains split across Pool and DVE so
#     they never queue behind the division chain. Host sums the 4 partials
#     per batch and transposes back.
import sys
import os

sys.path.insert(0, "/opt/trn_rl_repo")

import numpy as np

import concourse.bass as bass  # noqa: F401  (bass types used via bacc)
import concourse.mybir as mybir
from concourse import bacc
from concourse.tile import TileContext
from concourse.bass_utils import run_bass_kernel_spmd
from contextlib import ExitStack

F32 = mybir.dt.float32
BF16 = mybir.dt.bfloat16
AF = mybir.ActivationFunctionType
ALU = mybir.AluOpType

B, T, D = 2, 2048, 1024
H, HD = 16, 64
NCORES = 8
GROUPS = NCORES // B          # cores per batch = 4
HPC = H // GROUPS             # heads per core = 4
NK = D // 128                 # contraction tiles for D
SCALE = HD ** -0.5

# hd interleave: new row 2j <- orig j, new row 2j+1 <- orig j+32 so the
# rotate-half partner of every row is its neighbour (swappable by a 32-lane
# stream shuffle).
PI = np.empty(HD, dtype=np.int64)
PI[0::2] = np.arange(32)
PI[1::2] = np.arange(32, 64)

SWAP_MASK = []
for _i in range(16):
    SWAP_MASK += [2 * _i + 1, 2 * _i]

# w_cat column tiles (host order): c0=q pair0, c1=k pair0 (first so pair-0
# attention inputs drain earliest, and the first w DMA can cover just c0/c1),
# c2/c3 = v pairs, c4=q pair1, c5=k pair1.
ROPE_DST = {0: ("q", 0), 1: ("k", 0), 4: ("q", 1), 5: ("k", 1)}


def _build_program():
    nc = bacc.Bacc("TRN2", target_bir_lowering=False, debug=False,
                   num_devices=NCORES)
    d_x = nc.dram_tensor("xT", [D, T], BF16, kind="ExternalInput").ap()
    d_w = nc.dram_tensor("w_cat", [D, 6 * 128], BF16,
                         kind="ExternalInput").ap()
    d_wo = nc.dram_tensor("w_o", [2 * 128, D], BF16,
                          kind="ExternalInput").ap()
    d_cos = nc.dram_tensor("cos2", [128, T], BF16, kind="ExternalInput").ap()
    d_sin = nc.dram_tensor("sin2", [128, T], BF16, kind="ExternalInput").ap()
    d_id = nc.dram_tensor("ident", [128, 128], BF16,
                          kind="ExternalInput").ap()
    # [si*4+j, r, nn*512+q] blocks; host reassembles to [D, T]
    d_out = nc.dram_tensor("outp", [16, 128, 1024], BF16,
                           kind="ExternalOutput").ap()

    with TileContext(nc) as tc, nc.allow_low_precision(reason="bf16 attn"):
        with ExitStack() as root:
            qkv_pool = root.enter_context(tc.tile_pool(name="qkv", bufs=1))
            va_pool = root.enter_context(tc.tile_pool(name="va", bufs=1))
            out_pool = root.enter_context(tc.tile_pool(name="outT", bufs=1))
            wop = root.enter_context(tc.tile_pool(name="wop", bufs=1))
            wu_pool = root.enter_context(tc.tile_pool(name="wu", bufs=1))

            qT = [qkv_pool.tile([128, T], BF16, tag=f"q{p}", name=f"qT{p}")
                  for p in range(2)]
            kT = [qkv_pool.tile([128, T], BF16, tag=f"k{p}", name=f"kTt{p}")
                  for p in range(2)]
            va = [va_pool.tile([128, 16 * 65], BF16, tag=f"va{h}",
                               name=f"va{h}") for h in range(HPC)]
            oT = [out_pool.tile([128, T], BF16, tag=f"o{p}", name=f"oT{p}")
                  for p in range(2)]
            wo_sb = [wop.tile([128, D], BF16, tag=f"wo{p}", name=f"wo{p}")
                     for p in range(2)]

            # Warm the Act engine's exp table before it matters.
            wu = wu_pool.tile([1, 2], F32, tag="wu")
            wu2 = wu_pool.tile([1, 2], F32, tag="wu2")
            nc.vector.memset(wu[:], 0.0)
            nc.scalar.activation(wu2[:], wu[:], AF.Exp, scale=1.0)
            # denominator ones columns (free on Pool; avoids a strided DMA)
            for h in range(HPC):
                nc.gpsimd.memset(va[h][:, 64:16 * 65:65], 1.0)
            # causal-mask tile: -1e9 where col < partition, else 0. Added
            # into the S psum group via one tiny matmul (stationary =
            # identity) so exp zeroes the future positions exactly and no
            # per-block affine_select sits on the exp->AV critical path.
            tri = wu_pool.tile([128, 128], BF16, tag="tri")
            nc.gpsimd.memset(tri[:], -1e9)
            nc.gpsimd.affine_select(tri[:], tri[:], pattern=[[1, 128]],
                                    compare_op=ALU.is_lt, fill=0.0,
                                    base=0, channel_multiplier=-1)

            # ---------------- Phase A: qkv projection + RoPE + v transpose
            with nc.named_scope("qkv"):
                with ExitStack() as sA:
                    tab = sA.enter_context(tc.tile_pool(name="tab", bufs=1))
                    xp = sA.enter_context(tc.tile_pool(name="xp", bufs=1))
                    wp = sA.enter_context(tc.tile_pool(name="wp", bufs=1))
                    tp = sA.enter_context(tc.tile_pool(name="ropetmp",
                                                       bufs=2))
                    vtp = sA.enter_context(tc.tile_pool(name="vT", bufs=1))

                    cos2 = tab.tile([128, T], BF16, tag="cos")
                    sin2 = tab.tile([128, T], BF16, tag="sin")
                    ident = wu_pool.tile([128, 128], BF16, tag="id")
                    vT = [vtp.tile([128, T], BF16, tag=f"v{p}",
                                   name=f"vT{p}") for p in range(2)]
                    x_sb = [xp.tile([128, T], BF16, tag=f"x{kt}",
                                    name=f"xsb{kt}") for kt in range(NK)]
                    w_sb = [wp.tile([128, 6 * 128], BF16, tag=f"w{kt}",
                                    name=f"wsb{kt}") for kt in range(NK)]

                    # DMA queue: x(kt,t0) + w(kt) pairs stream first (the
                    # first w transfer covers only q0/k0 so the PE starts
                    # ~2us in); tables follow in consumption order.
                    for kt in range(NK):
                        nc.sync.dma_start(out=x_sb[kt][:, 0:512],
                                          in_=d_x[kt * 128:(kt + 1) * 128,
                                                  0:512])
                        if kt == 0:
                            nc.sync.dma_start(out=w_sb[0][:, 0:256],
                                              in_=d_w[0:128, 0:256])
                            nc.sync.dma_start(out=w_sb[0][:, 256:768],
                                              in_=d_w[0:128, 256:768])
                            nc.sync.dma_start(out=cos2[:, 0:512],
                                              in_=d_cos[:, 0:512])
                            nc.sync.dma_start(out=sin2[:, 0:512],
                                              in_=d_sin[:, 0:512])
                        else:
                            nc.sync.dma_start(
                                out=w_sb[kt][:],
                                in_=d_w[kt * 128:(kt + 1) * 128, :])
                    nc.sync.dma_start(out=cos2[:, 512:1024],
                                      in_=d_cos[:, 512:1024])
                    nc.sync.dma_start(out=sin2[:, 512:1024],
                                      in_=d_sin[:, 512:1024])
                    nc.sync.dma_start(out=ident[:], in_=d_id[:])
                    for kt in range(NK):
                        nc.sync.dma_start(
                            out=x_sb[kt][:, 512:2048],
                            in_=d_x[kt * 128:(kt + 1) * 128, 512:2048])
                    nc.sync.dma_start(out=cos2[:, 1024:2048],
                                      in_=d_cos[:, 1024:2048])
                    nc.sync.dma_start(out=sin2[:, 1024:2048],
                                      in_=d_sin[:, 1024:2048])
                    for p in range(2):
                        nc.sync.dma_start(
                            out=wo_sb[p][:],
                            in_=d_wo[p * 128:(p + 1) * 128, :])

                    accp = tc.alloc_tile_pool(name="accs", bufs=1,
                                              space="PSUM")
                    psT = tc.alloc_tile_pool(name="psT", bufs=2, space="PSUM",
                                             side="right")

                    def emit_rope(c, acc, tsl):
                        kind, pair = ROPE_DST[c]
                        dst = qT[pair] if kind == "q" else kT[pair]
                        qsb = tp.tile([128, 512], BF16, tag="qsb")
                        nc.gpsimd.tensor_copy(qsb[:], acc[:])
                        qsh = tp.tile([128, 512], BF16, tag="qsh")
                        nc.vector.stream_shuffle(qsh[:], acc[:], SWAP_MASK)
                        tcos = tp.tile([128, 512], BF16, tag="tcos")
                        nc.vector.tensor_tensor(out=tcos[:], in0=qsb[:],
                                                in1=cos2[:, tsl],
                                                op=ALU.mult)
                        nc.vector.tensor_tensor(out=qsh[:], in0=qsh[:],
                                                in1=sin2[:, tsl],
                                                op=ALU.mult)
                        nc.vector.tensor_tensor(out=dst[:, tsl], in0=tcos[:],
                                                in1=qsh[:], op=ALU.add)

                    def emit_vtrans(t):
                        # transposes + va copies for both v c-tiles of chunk
                        # t; the t=3 copies go to DVE (idle entering
                        # attention) so Pool carries no backlog there
                        eng = nc.vector if t == 3 else nc.gpsimd
                        for p in range(2):
                            pt_ = psT.tile([128, 512], BF16, tag="pt",
                                           name=f"ptr{p}_{t}")
                            for j in range(4):
                                tt = 4 * t + j
                                nc.tensor.transpose(
                                    pt_[:, j * 128:(j + 1) * 128],
                                    vT[p][:, tt * 128:(tt + 1) * 128],
                                    ident[:])
                            for j in range(4):
                                tt = 4 * t + j
                                eng.tensor_copy(
                                    va[2 * p][:, tt * 65:tt * 65 + 64],
                                    pt_[:, j * 128:j * 128 + 64])
                                eng.tensor_copy(
                                    va[2 * p + 1][:, tt * 65:tt * 65 + 64],
                                    pt_[:, j * 128 + 64:j * 128 + 128])

                    def drain(c, acc, tsl):
                        if c in (2, 3):
                            nc.scalar.copy(vT[c - 2][:, tsl], acc[:])
                        else:
                            emit_rope(c, acc, tsl)

                    for t in range(4):
                        tsl = slice(t * 512, (t + 1) * 512)
                        accs = [accp.tile([128, 512], F32, tag=f"a{c}",
                                          name=f"acc{c}_{t}")
                                for c in range(6)]
                        if t < 2:
                            # contraction-tile inner: matches the x DMA pace
                            for kt in range(NK):
                                for c in range(6):
                                    nc.tensor.matmul(
                                        accs[c][:],
                                        w_sb[kt][:, c * 128:(c + 1) * 128],
                                        x_sb[kt][:, tsl],
                                        start=(kt == 0), stop=(kt == NK - 1))
                            if t == 1:
                                emit_vtrans(0)
                            for c in range(6):
                                drain(c, accs[c], tsl)
                        else:
                            # output-major with immediate drains: PSUM banks
                            # free progressively, so attention starts with no
                            # bubble after t=3.
                            for c in range(6):
                                for kt in range(NK):
                                    nc.tensor.matmul(
                                        accs[c][:],
                                        w_sb[kt][:, c * 128:(c + 1) * 128],
                                        x_sb[kt][:, tsl],
                                        start=(kt == 0), stop=(kt == NK - 1))
                                if c == 5:
                                    emit_vtrans(t - 1)
                                drain(c, accs[c], tsl)
                    emit_vtrans(3)
                    psT.release()
                    accp.release()

            # ---------------- attention + interleaved out-projection
            psS = tc.alloc_tile_pool(name="psS", bufs=2, space="PSUM")
            psV = tc.alloc_tile_pool(name="psV", bufs=2, space="PSUM",
                                     side="right")

            with nc.named_scope("attn"):
                with ExitStack() as sB:
                    ptp = sB.enter_context(tc.tile_pool(name="ptp", bufs=6))
                    rp = sB.enter_context(tc.tile_pool(name="rp", bufs=2))
                    fop = sB.enter_context(tc.tile_pool(name="fop", bufs=4))

                    # oproj chunks of the previous strip, sprinkled between
                    # attention units to fill the PE while the Act engine
                    # (the attention bottleneck) catches up on exp
                    oproj_q = []

                    def emit_oproj_chunk(si, j):
                        q0 = 512 * si
                        pD = psS.tile([128, 1024], F32, tag="sps",
                                      name=f"pD{si}_{j}")
                        for nn in range(2):
                            n = 2 * j + nn
                            for p in range(2):
                                nc.tensor.matmul(
                                    pD[:, nn * 512:(nn + 1) * 512],
                                    wo_sb[p][:, n * 128:(n + 1) * 128],
                                    oT[p][:, q0:q0 + 512],
                                    start=(p == 0), stop=(p == 1))
                        fo = fop.tile([128, 1024], BF16, tag="fo",
                                      name=f"fo{si}_{j}")
                        nc.vector.tensor_copy(fo[:], pD[:])
                        nc.sync.dma_start(out=d_out[4 * si + j], in_=fo[:])

                    def emit_strip(si, p):
                        q0 = 512 * si
                        kb_max = 4 * (si + 1)
                        av = psV.tile([65, 1024], F32, tag="av",
                                      name=f"avps{si}_{p}")
                        # units: fully-causal kb pairs (one exp per head),
                        # then the 4 diagonal blocks individually
                        units = [("pair", kb) for kb in range(0, 4 * si, 2)]
                        units += [("diag", kb) for kb in range(4 * si,
                                                               kb_max)]
                        pend = []

                        def emit_s(unit):
                            kind, kb = unit
                            if kind == "pair":
                                ptbs = []
                                for hl in range(2):
                                    hb = 64 * hl
                                    sps = psS.tile(
                                        [128, 1024], F32, tag="sps",
                                        name=f"sp{si}_{p}_{kb}_{hl}")
                                    for dk in range(2):
                                        nc.tensor.matmul(
                                            sps[:, dk * 512:(dk + 1) * 512],
                                            kT[p][hb:hb + 64,
                                                  (kb + dk) * 128:
                                                  (kb + dk + 1) * 128],
                                            qT[p][hb:hb + 64, q0:q0 + 512],
                                            start=True, stop=True)
                                    ptb = ptp.tile(
                                        [128, 1024], BF16, tag="ptb",
                                        name=f"pt{si}_{p}_{kb}_{hl}")
                                    nc.scalar.activation(
                                        ptb[:], sps[:], AF.Exp, scale=SCALE)
                                    ptbs.append(ptb)
                                return ptbs
                            # diagonal block: both heads in one sps tile;
                            # the tri matmul adds -1e9 to future positions
                            # inside the psum group, so exp masks for free
                            o = 128 * kb - q0
                            L = 512 - o
                            sps = psS.tile([128, 1024], F32, tag="sps",
                                           name=f"sp{si}_{p}_{kb}")
                            for hl in range(2):
                                hb = 64 * hl
                                nc.tensor.matmul(
                                    sps[:, 512 * hl + o:512 * hl + 512],
                                    kT[p][hb:hb + 64,
                                          kb * 128:(kb + 1) * 128],
                                    qT[p][hb:hb + 64, q0 + o:q0 + 512],
                                    start=True, stop=False,
                                    skip_group_check=True)
                                nc.tensor.matmul(
                                    sps[:, 512 * hl + o:512 * hl + o + 128],
                                    ident[:], tri[:],
                                    start=False, stop=True,
                                    skip_group_check=True)
                            ptb = ptp.tile([128, 1024], BF16, tag="ptb",
                                           name=f"pt{si}_{p}_{kb}")
                            sps3 = sps[:].rearrange("a (h q) -> a h q", h=2)
                            ptb3 = ptb[:].rearrange("a (h q) -> a h q", h=2)
                            nc.scalar.activation(
                                ptb3[:, :, 0:L], sps3[:, :, o:512],
                                AF.Exp, scale=SCALE)
                            return ptb

                        def emit_av(unit, ptbs):
                            kind, kb = unit
                            if kind == "pair":
                                for hl in range(2):
                                    for dk in range(2):
                                        nc.tensor.matmul(
                                            av[:, hl * 512:(hl + 1) * 512],
                                            va[2 * p + hl][
                                                :, (kb + dk) * 65:
                                                (kb + dk) * 65 + 65],
                                            ptbs[hl][:, dk * 512:
                                                     (dk + 1) * 512],
                                            start=(kb + dk == 0),
                                            stop=(kb + dk == kb_max - 1),
                                            skip_group_check=True)
                            else:
                                o = 128 * kb - q0
                                L = 512 - o
                                for hl in range(2):
                                    nc.tensor.matmul(
                                        av[:, hl * 512 + o:hl * 512 + 512],
                                        va[2 * p + hl][:, kb * 65:
                                                       kb * 65 + 65],
                                        ptbs[:, 512 * hl:512 * hl + L],
                                        start=(kb == 0),
                                        stop=(kb == kb_max - 1),
                                        skip_group_check=True)

                        for i, unit in enumerate(units):
                            pend.append((unit, emit_s(unit)))
                            if len(pend) > 1:
                                emit_av(*pend.pop(0))
                            if i % 3 == 2 and oproj_q:
                                emit_oproj_chunk(*oproj_q.pop(0))
                        while pend:
                            emit_av(*pend.pop(0))
                        # division, split per head to halve chain latency:
                        # row 64 of av is the denominator
                        for hl in range(2):
                            hsl = slice(hl * 512, (hl + 1) * 512)
                            r_sb = rp.tile([1, 512], F32, tag="r",
                                           name=f"rsb{si}_{p}_{hl}")
                            nc.vector.reciprocal(r_sb[:], av[64:65, hsl])
                            rb = rp.tile([64, 512], F32, tag="rb",
                                         name=f"rbb{si}_{p}_{hl}")
                            nc.gpsimd.partition_broadcast(rb[:], r_sb[:])
                            nc.vector.tensor_tensor(
                                out=oT[p][64 * hl:64 * hl + 64,
                                          q0:q0 + 512],
                                in0=av[0:64, hsl], in1=rb[:],
                                op=ALU.mult)

                    # strip order 3,0,1,2: each strip's out-projection is
                    # sprinkled through the following strip, and the final
                    # strip's division hides under the second-to-last oproj
                    for i, si in enumerate([3, 0, 1, 2]):
                        emit_strip(si, 0)
                        emit_strip(si, 1)
                        oproj_q += [(si, j) for j in range(4)]
                    while oproj_q:
                        emit_oproj_chunk(*oproj_q.pop(0))

            psS.release()
            psV.release()

    nc.compile()
    return nc


_NC_CACHE = None


def _get_program():
    global _NC_CACHE
    if _NC_CACHE is None:
        _NC_CACHE = _build_program()
    return _NC_CACHE


def _rope_tables():
    inv_freq = 1.0 / (10000.0 ** (np.arange(0, HD, 2, dtype=np.float32) / HD))
    freqs = np.outer(np.arange(T, dtype=np.float32), inv_freq)  # [T, 32]
    emb = np.concatenate([freqs, freqs], axis=-1)               # [T, 64]
    return np.cos(emb), np.sin(emb)


def _to_bf16(a):
    import ml_dtypes
    return np.asarray(a, dtype=np.float32).astype(ml_dtypes.bfloat16)


def _host_prep(x, w_qkv, w_out):
    cos, sin = _rope_tables()          # [T, 64] each, original hd order
    # permuted + transposed tables [64, T], duplicated for a 2-head pair tile
    cosP = np.ascontiguousarray(cos.T[PI, :])                   # [64, T]
    sinP = sin.T[PI, :].copy()                                  # [64, T]
    sinP[0::2, :] *= -1.0                                       # sign baked in
    cos2 = _to_bf16(np.vstack([cosP, cosP]))
    sin2 = _to_bf16(np.vstack([sinP, sinP]))
    ident = _to_bf16(np.eye(128, dtype=np.float32))

    in_maps = []
    for core in range(NCORES):
        b = core // GROUPS
        h0 = (core % GROUPS) * HPC
        xT = np.ascontiguousarray(x[b].T)                       # [D, T]
        # column order: q pair0, k pair0, v pair0, v pair1, q pair1, k pair1
        cols = []
        for kind, p in [(0, 0), (1, 0)]:                        # q0, k0
            for hh in range(2):
                h = h0 + 2 * p + hh
                wcol = w_qkv[:, kind * D + h * HD:kind * D + (h + 1) * HD]
                cols.append(wcol[:, PI])
        for p in range(2):                                      # v (no perm)
            for hh in range(2):
                h = h0 + 2 * p + hh
                cols.append(w_qkv[:, 2 * D + h * HD:2 * D + (h + 1) * HD])
        for kind, p in [(0, 1), (1, 1)]:                        # q1, k1
            for hh in range(2):
                h = h0 + 2 * p + hh
                wcol = w_qkv[:, kind * D + h * HD:kind * D + (h + 1) * HD]
                cols.append(wcol[:, PI])
        w_cat = np.concatenate(cols, axis=1)                    # [D, 768]
        w_o = w_out[h0 * HD:(h0 + HPC) * HD, :]                 # [256, D]
        in_maps.append({
            "xT": _to_bf16(xT),
            "w_cat": _to_bf16(w_cat),
            "w_o": _to_bf16(w_o),
            "cos2": cos2,
            "sin2": sin2,
            "ident": ident,
        })
    return in_maps


def kernel(x, w_qkv, w_out):
    x = np.asarray(x, dtype=np.float32)
    w_qkv = np.asarray(w_qkv, dtype=np.float32)
    w_out = np.asarray(w_out, dtype=np.float32)
    nc = _get_program()
    in_maps = _host_prep(x, w_qkv, w_out)
    trace = bool(int(os.environ.get("KBENCH_TRACE", "0")))
    res = run_bass_kernel_spmd(nc, in_maps, list(range(NCORES)), trace=trace)
    if trace and res.exec_time_ns is not None:
        print(f"HW exec time: {res.exec_time_ns} ns")
    out = np.zeros((B, T, D), dtype=np.float32)
    for core in range(NCORES):
        b = core // GROUPS
        blk = res.results[core]["outp"].astype(np.float32)
        # (si, j, r, nn, q) -> rows (j,nn,r) = D, cols (si,q) = T
        dT = blk.reshape(4, 4, 128, 2, 512).transpose(1, 3, 2, 0, 4)
        out[b] += dT.reshape(D, T).T
    return out
